# revision 1
# baseline (speedup 1.0000x reference)
"""Trainium2 Bass kernel for nn_CausalSelfAttention_74268574482879.

The reference module's attention scores are overwritten by the causal mask
(q/k are discarded), so softmax weights are uniform over positions <= t:
    y = cummean_T(x) @ W_v @ W_p,   W_v = w_attn[:, 1024:1536]

Distribution: the 4096 rows of (B*T) are split into 8 chunks of 512 rows,
one per NeuronCore.  The only cross-chunk dependency is the column-sum of
all preceding rows in the same batch element; the host passes that tiny
(512,) halo vector per core while slicing the shards.

Per-core dataflow (matmuls keep operands in natural layout — the PE's
implicit transpose of the stationary operand does all layout work):
  stage A: lhsT=x_tile, rhs=U_scaled  ->  psA = scale*(local cumsum)^T (PSUM)
           tile colsums are recovered from psA's last column (one fused
           tensor_scalar each), then a DVE/GpSimd carry adds P_j[c]*scale[t]
  stage B: lhsT=Wv,     rhs=A^T       ->  M1^T = (A @ Wv)^T
  stage C: lhsT=M1^T,   rhs=Wp        ->  Y = M1 @ Wp   (natural, DMA out)
"""

import numpy as np

import concourse.bass as bass
import concourse.bacc as bacc
import concourse.mybir as mybir
import concourse.tile as tile
from concourse import bass_utils

N_CORES = 8
B, T, C = 2, 2048, 512
CHUNK = 512               # rows of flattened (B*T) per core
P = 128
NT = CHUNK // P           # 4 row-tiles per chunk
NI = C // P               # 4 col-tiles of the 512 feature dim
F32 = mybir.dt.float32
F32R = mybir.dt.float32r
BF16 = mybir.dt.bfloat16

MODE = ["f32r"]           # "f32" | "f32r" | "bf16" (stage B/C dtype)
TRACE = [False]
LAST_RESULT = [None]
_STATE = {}


def _build_nc(mode):
    nc = bacc.Bacc(
        "TRN2", target_bir_lowering=False, debug=False, num_devices=N_CORES
    )
    # dtype plan per mode: a_dt feeds stage-A matmuls, bc_dt feeds B/C.
    # float32r keeps fp32 bits but runs the PE in single-pass reduced mode;
    # the verifier wants every producer of a matmul operand to declare it.
    if mode == "f32":
        a_dt, bc_dt = F32, F32
    elif mode == "f32r":
        a_dt, bc_dt = F32R, F32R
    else:  # bf16 B/C, f32r stage A
        a_dt, bc_dt = F32R, BF16
    bc_bf16 = bc_dt == BF16
    wdma_dt = F32 if bc_bf16 else bc_dt

    x_d = nc.dram_tensor("x", (CHUNK, C), a_dt, kind="ExternalInput")
    wv_d = nc.dram_tensor("wv", (C, C), wdma_dt, kind="ExternalInput")
    wp_d = nc.dram_tensor("wp", (C, C), wdma_dt, kind="ExternalInput")
    us_d = nc.dram_tensor("us", (P, P), a_dt, kind="ExternalInput")
    sc_d = nc.dram_tensor("sc", (P, NI + NT), F32, kind="ExternalInput")
    y_d = nc.dram_tensor("y", (CHUNK, C), F32, kind="ExternalOutput")

    x_ap, wv_ap, wp_ap = x_d.ap(), wv_d.ap(), wp_d.ap()
    us_ap, sc_ap, y_ap = us_d.ap(), sc_d.ap(), y_d.ap()

    with tile.TileContext(nc) as tc:
        with (
            tc.tile_pool(name="io", bufs=1) as io,
            tc.tile_pool(name="tmp", bufs=4) as tmp_pool,
            tc.tile_pool(name="psbig", bufs=2, space="PSUM") as ps_pool,
        ):
            # ---- inputs to SBUF (order = DMA priority) ----
            # x arrives as column-slices aligned to the i-rounds: slice i
            # holds all 512 rows of features ci as (P, NT, P)
            x_r = x_ap.rearrange("(j p) c -> p j c", p=P)
            us_sb = io.tile([P, P], a_dt, name="us_sb")
            nc.gpsimd.dma_start(us_sb[:], us_ap[:, :])
            xc = []
            for i in range(NI):
                t = io.tile([P, NT, P], a_dt, name=f"xc{i}")
                eng = nc.sync if i % 2 == 0 else nc.gpsimd
                eng.dma_start(t[:], x_r[:, :, i * P : (i + 1) * P])
                xc.append(t)
                if i == 0:
                    # pc | scv: prefix colsums and the final 1/(t+1) column
                    cs_sb = io.tile([P, NI + NT], F32, name="cs_sb")
                    nc.sync.dma_start(cs_sb[:], sc_ap[:, :])
            pc_sb = cs_sb[:, 0:NI]
            scv_sb = cs_sb[:, NI : NI + NT]
            wv_pack = io.tile([P, NI, C], wdma_dt, name="wv_pack")
            nc.sync.dma_start(wv_pack[:], wv_ap.rearrange("(k p) c -> p k c", p=P))
            wp_pack = io.tile([P, NI, C], wdma_dt, name="wp_pack")
            nc.sync.dma_start(wp_pack[:], wp_ap.rearrange("(k p) c -> p k c", p=P))
            wv_sb = [wv_pack[:, i, :] for i in range(NI)]
            wp_sb = [wp_pack[:, j, :] for j in range(NI)]

            if bc_bf16:
                wvb, wpb = [], []
                for i in range(NI):
                    t = io.tile([P, C], BF16, name=f"wvb{i}")
                    nc.scalar.copy(t[:], wv_sb[i][:])
                    wvb.append(t)
                for j in range(NI):
                    t = io.tile([P, C], BF16, name=f"wpb{j}")
                    nc.scalar.copy(t[:], wp_sb[j][:])
                    wpb.append(t)
            else:
                wvb, wpb = wv_sb, wp_sb

            # ---- stage A: raw local cumsum; i-outer rounds so A_sb[i]
            # completes early and stage B overlaps.  The 1/(t+1) scale is
            # deferred all the way to the Y eviction (it commutes) ----
            Pc_sb = io.tile([P, NT * NI], F32, name="Pc_sb")
            A_sb = [
                io.tile([P, CHUNK], bc_dt, name=f"A{i}") for i in range(NI)
            ]
            for i in range(NI):
                nc.vector.tensor_copy(
                    Pc_sb[:, i * NT : i * NT + 1], pc_sb[:, i : i + 1]
                )
                psA = []
                for j in range(NT):
                    pa = ps_pool.tile(
                        [P, P], F32, name=f"psA{i}_{j}", tag="small", bufs=6
                    )
                    nc.tensor.matmul(
                        pa[:], xc[i][:, j, :], us_sb[:], start=True, stop=True
                    )
                    psA.append(pa)
                for j in range(NT):
                    col = i * NT + j
                    if j + 1 < NT:
                        # running prefix: next = cur + colsum_j (psA last col)
                        nc.vector.tensor_add(
                            Pc_sb[:, col + 1 : col + 2],
                            Pc_sb[:, col : col + 1],
                            psA[j][:, P - 1 : P],
                        )
                    nc.vector.tensor_scalar_add(
                        A_sb[i][:, j * P : (j + 1) * P],
                        psA[j][:],
                        Pc_sb[:, col : col + 1],
                    )

            # ---- stage B: M1^T = (A @ Wv)^T ----
            M1_sb = []
            for jj in range(NI):
                psm = ps_pool.tile([P, CHUNK], F32, name=f"psM{jj}", tag="big")
                cj = slice(jj * P, (jj + 1) * P)
                for i in range(NI):
                    nc.tensor.matmul(
                        psm[:],
                        wvb[i][:, cj],
                        A_sb[i][:],
                        start=(i == 0),
                        stop=(i == NI - 1),
                    )
                m1 = io.tile([P, CHUNK], bc_dt, name=f"M1{jj}")
                nc.vector.tensor_copy(m1[:], psm[:])
                M1_sb.append(m1)

            # ---- stage C: Y = M1 @ Wp  (natural layout) ----
            for tt in range(NT):
                psy = ps_pool.tile([P, C], F32, name=f"psY{tt}", tag="big")
                st = slice(tt * P, (tt + 1) * P)
                for jj in range(NI):
                    nc.tensor.matmul(
                        psy[:],
                        M1_sb[jj][:, st],
                        wpb[jj][:],
                        start=(jj == 0),
                        stop=(jj == NI - 1),
                    )
                ysb = io.tile([P, C], F32, name=f"y{tt}")
                nc.vector.tensor_scalar_mul(
                    ysb[:], psy[:], scv_sb[:, tt : tt + 1]
                )
                nc.sync.dma_start(y_ap[st, :], ysb[:])

    nc.compile()
    return nc


def _get_nc():
    key = MODE[0]
    if key not in _STATE:
        _STATE[key] = _build_nc(key)
    return _STATE[key]


def _prepare_in_maps(x, w_attn, w_proj):
    x = np.asarray(x, dtype=np.float32)
    w_attn = np.asarray(w_attn, dtype=np.float32)
    w_proj = np.ascontiguousarray(np.asarray(w_proj, dtype=np.float32))
    wv = np.ascontiguousarray(w_attn[:, 2 * C : 3 * C])

    in_maps = []
    for core in range(N_CORES):
        b, tc = divmod(core, T // CHUNK)
        goff = tc * CHUNK
        chunk = np.ascontiguousarray(x[b, goff : goff + CHUNK, :])
        # halo: column-sum of all earlier rows in this batch element
        p = x[b, :goff, :].sum(axis=0, dtype=np.float32) if goff else np.zeros(
            C, np.float32
        )
        # scv[r, tt] = 1/(global_row+1) for row tt*P + r of this chunk
        scale = (1.0 / (goff + np.arange(1, CHUNK + 1))).astype(np.float32)
        sc = np.concatenate(
            [p.reshape(NI, P).T, scale.reshape(NT, P).T], axis=1
        ).astype(np.float32)
        us = np.triu(np.ones((P, P), np.float32))  # s <= t
        in_maps.append(
            {"x": chunk, "wv": wv, "wp": w_proj, "us": us, "sc": sc}
        )
    return in_maps


def kernel(x, w_attn, w_proj):
    nc = _get_nc()
    in_maps = _prepare_in_maps(x, w_attn, w_proj)
    res = bass_utils.run_bass_kernel_spmd(
        nc, in_maps, core_ids=list(range(N_CORES)), trace=TRACE[0]
    )
    LAST_RESULT[0] = res
    y = np.empty((B, T, C), np.float32)
    for core in range(N_CORES):
        b, tc = divmod(core, T // CHUNK)
        y[b, tc * CHUNK : (tc + 1) * CHUNK, :] = res.results[core]["y"]
    return y



# revision 2
# speedup vs baseline: 1.3250x; 1.3250x over previous
"""Trainium2 Bass kernel for nn_CausalSelfAttention_74268574482879.

The reference module's attention scores are overwritten by the causal mask
(q/k are discarded), so softmax weights are uniform over positions <= t:
    y = cummean_T(x) @ W_v @ W_p

Host-side algebra (all exact up to fp rounding):
  * W_c = W_v @ W_p is folded into a single 512x512 weight.
  * The 4096 rows of (B*T) are split into 8 chunks of 512 rows, one per
    NeuronCore.  The cross-chunk carry (column-sum of all preceding rows in
    the same batch element) is added into row 0 of each chunk on the host,
    so the device computes a plain local cumsum.
  * Everything is cast to bf16 on the host (rel-err budget is 2e-2).

Per-core dataflow:
  stage A: A^T_i = cumsum(x)^T feature-slice i, via block-triangular
           matmuls: lhsT = x row-tile feature slice, rhs = [triu|1|1|1]
           streaming operand.  16 matmuls, no carry chain.
  stage M: psY_j = A[tile j] @ W_c (natural layout), 16 matmuls; the
           1/(t+1) row scale is fused into the PSUM->SBUF eviction
           (per-partition tensor_scalar on DVE / scaled copy on ACT).
A few throwaway matmuls on memset data run during the initial DMA fill to
lift the PE HAM clock-gate early.
"""

import numpy as np
import ml_dtypes

import concourse.bass as bass
import concourse.bacc as bacc
import concourse.mybir as mybir
import concourse.tile as tile
from concourse import bass_utils

N_CORES = 8
B, T, C = 2, 2048, 512
CHUNK = 512               # rows of flattened (B*T) per core
P = 128
NT = CHUNK // P           # 4 row-tiles per chunk
NI = C // P               # 4 col-tiles of the 512 feature dim
F32 = mybir.dt.float32
BF16 = mybir.dt.bfloat16
BF16_NP = ml_dtypes.bfloat16

N_WARM = [4]              # warmup matmuls (HAM unthrottle) during DMA fill
TRACE = [False]
LAST_RESULT = [None]
_STATE = {}


def _build_nc(n_warm):
    nc = bacc.Bacc(
        "TRN2", target_bir_lowering=False, debug=False, num_devices=N_CORES
    )

    x_d = nc.dram_tensor("x", (CHUNK, C), BF16, kind="ExternalInput")
    wc_d = nc.dram_tensor("wc", (C, C), BF16, kind="ExternalInput")
    us_d = nc.dram_tensor("us", (P, P), BF16, kind="ExternalInput")
    sc_d = nc.dram_tensor("sc", (P, NT), F32, kind="ExternalInput")
    y_d = nc.dram_tensor("y", (CHUNK, C), BF16, kind="ExternalOutput")

    x_ap, wc_ap, us_ap, sc_ap, y_ap = (
        x_d.ap(), wc_d.ap(), us_d.ap(), sc_d.ap(), y_d.ap()
    )

    with tile.TileContext(nc) as tc:
        with (
            tc.tile_pool(name="io", bufs=1) as io,
            tc.tile_pool(name="ps", bufs=1, space="PSUM") as ps_pool,
        ):
            # ---- warmup: keep PE busy from t=0 so HAM unthrottles ----
            warm = io.tile([P, C], BF16, name="warm")
            nc.vector.memset(warm[:], 0.0)
            if n_warm:
                wps = ps_pool.tile([P, C], F32, name="wps", tag="warm")
                for _ in range(n_warm):
                    nc.tensor.matmul(
                        wps[:], warm[:, 0:P], warm[:], start=True, stop=True
                    )

            # ---- inputs (issue order = DMA priority) ----
            # uo = [triu | ones ones ones]: memset 1.0, overwrite cols 0:128
            uo = io.tile([P, NT * P], BF16, name="uo")
            nc.gpsimd.memset(uo[:], 1.0)
            nc.sync.dma_start(uo[:, 0:P], us_ap[:, :])
            scs = io.tile([P, NT], F32, name="scs")
            nc.sync.dma_start(scs[:], sc_ap[:, :])
            # x as (p, j, c): row tile j on partitions, 1KB runs
            xs = io.tile([P, NT, C], BF16, name="xs")
            nc.sync.dma_start(xs[:], x_ap.rearrange("(j p) c -> p j c", p=P))
            wcs = io.tile([P, NI, C], BF16, name="wcs")
            nc.scalar.dma_start(wcs[:], wc_ap.rearrange("(k p) c -> p k c", p=P))

            # ---- stages A and M, interleaved for PE/eviction overlap ----
            # A^T_i: [feat slice i on partitions, 512 rows], bf16 in SBUF
            A_sb = [io.tile([P, CHUNK], BF16, name=f"A{i}") for i in range(NI)]
            psY = [
                ps_pool.tile([P, C], F32, name=f"psY{j}", tag=f"Y{j}")
                for j in range(NT)
            ]

            def stage_a(i):
                pa = ps_pool.tile([P, CHUNK], F32, name=f"psA{i}", tag="A", bufs=2)
                for j in range(NT):
                    # out rows j*P.. get tri-block from x_j plus ones-blocks
                    nc.tensor.matmul(
                        pa[:, j * P : CHUNK],
                        xs[:, j, i * P : (i + 1) * P],
                        uo[:, 0 : (NT - j) * P],
                        start=(j == 0),
                        stop=(j == NT - 1),
                    )
                if i % 2 == 0:
                    nc.vector.tensor_copy(A_sb[i][:], pa[:])
                else:
                    nc.scalar.copy(A_sb[i][:], pa[:])

            def stage_m(i):
                for j in range(NT):
                    nc.tensor.matmul(
                        psY[j][:],
                        A_sb[i][:, j * P : (j + 1) * P],
                        wcs[:, i, :],
                        start=(i == 0),
                        stop=(i == NI - 1),
                    )

            # PE order: A0 A1 M0 A2 M1 A3 M2 M3 — each M_i only needs A_i's
            # eviction, which overlaps the next A's matmuls.
            stage_a(0)
            stage_a(1)
            stage_m(0)
            stage_a(2)
            stage_m(1)
            stage_a(3)
            stage_m(2)
            stage_m(3)

            # ---- scaled eviction + output ----
            for j in range(NT):
                ysb = io.tile([P, C], BF16, name=f"y{j}")
                if j % 2 == 0:
                    nc.vector.tensor_scalar_mul(ysb[:], psY[j][:], scs[:, j : j + 1])
                else:
                    nc.scalar.mul(ysb[:], psY[j][:], scs[:, j : j + 1])
                nc.sync.dma_start(y_ap[j * P : (j + 1) * P, :], ysb[:])

    nc.compile()
    return nc


def _get_nc():
    key = N_WARM[0]
    if key not in _STATE:
        _STATE[key] = _build_nc(key)
    return _STATE[key]


def _prepare_in_maps(x, w_attn, w_proj):
    x = np.asarray(x, dtype=np.float32)
    w_attn = np.asarray(w_attn, dtype=np.float32)
    w_proj = np.asarray(w_proj, dtype=np.float32)
    wc = (w_attn[:, 2 * C : 3 * C] @ w_proj).astype(BF16_NP)
    us = np.triu(np.ones((P, P), np.float32)).astype(BF16_NP)

    in_maps = []
    for core in range(N_CORES):
        b, tc_ = divmod(core, T // CHUNK)
        goff = tc_ * CHUNK
        chunk = np.array(x[b, goff : goff + CHUNK, :], dtype=np.float32)
        if goff:
            # fold the carry into row 0 (cumsum then includes it everywhere)
            chunk[0] += x[b, :goff, :].sum(axis=0, dtype=np.float32)
        sc = (
            1.0 / (goff + 1 + np.arange(CHUNK, dtype=np.float32))
        ).reshape(NT, P).T.astype(np.float32)
        in_maps.append(
            {
                "x": chunk.astype(BF16_NP),
                "wc": wc,
                "us": us,
                "sc": np.ascontiguousarray(sc),
            }
        )
    return in_maps


def kernel(x, w_attn, w_proj):
    nc = _get_nc()
    in_maps = _prepare_in_maps(x, w_attn, w_proj)
    res = bass_utils.run_bass_kernel_spmd(
        nc, in_maps, core_ids=list(range(N_CORES)), trace=TRACE[0]
    )
    LAST_RESULT[0] = res
    y = np.empty((B, T, C), np.float32)
    for core in range(N_CORES):
        b, tc_ = divmod(core, T // CHUNK)
        y[b, tc_ * CHUNK : (tc_ + 1) * CHUNK, :] = res.results[core]["y"].astype(
            np.float32
        )
    return y


# revision 6
# speedup vs baseline: 1.6011x; 1.2084x over previous
"""Trainium2 Bass kernel for nn_CausalSelfAttention_74268574482879.

The reference module's attention scores are overwritten by the causal mask
(q/k are discarded), so softmax weights are uniform over positions <= t:
    y = cummean_T(x) @ W_v @ W_p

Host-side algebra (all exact up to fp rounding):
  * W_c = W_v @ W_p is folded into a single 512x512 weight.
  * The 4096 rows of (B*T) are split into 8 chunks of 512 rows, one per
    NeuronCore.  The cross-chunk carry (column-sum of all preceding rows in
    the same batch element) is added into column 0 of the transposed chunk
    on the host, so the device computes a plain local cumsum.
  * x is passed TRANSPOSED (feature-major) so the cumsum runs as a DVE /
    GpSimd ``tensor_tensor_scan`` along the free (time) dim — no PE work.
  * Everything is cast to bf16 on the host (rel-err budget is 2e-2).

Per-core dataflow:
  scan_i : A^T_i[f, t] = cumsum_t(xT_i[f, t])   (DVE/GpSimd, bf16 out)
  M      : psY_j = sum_i A_i[tile j]^T-slice @ W_c rows i   (16 matmuls)
  evict  : ysb_j = psY_j * 1/(t+1)  (per-partition scalar, DVE/ACT/GpSimd)
A few throwaway matmuls on memset data run during the initial DMA fill to
lift the PE HAM clock-gate early.
"""

import numpy as np
import ml_dtypes

import concourse.bass as bass
import concourse.bacc as bacc
import concourse.mybir as mybir
import concourse.tile as tile
from concourse import bass_utils

N_CORES = 8
B, T, C = 2, 2048, 512
CHUNK = 512               # rows of flattened (B*T) per core
P = 128
NT = CHUNK // P           # 4 row-tiles per chunk
NI = C // P               # 4 col-tiles of the 512 feature dim
F32 = mybir.dt.float32
BF16 = mybir.dt.bfloat16
BF16_NP = ml_dtypes.bfloat16
ADD = mybir.AluOpType.add
BYPASS = mybir.AluOpType.bypass

N_WARM = [5]              # warmup matmuls (HAM unthrottle) during DMA fill
TRACE = [False]
LAST_RESULT = [None]
_STATE = {}


def _build_nc(n_warm):
    nc = bacc.Bacc(
        "TRN2", target_bir_lowering=False, debug=False, num_devices=N_CORES
    )

    xt_d = nc.dram_tensor("xt", (C, CHUNK), BF16, kind="ExternalInput")
    wc_d = nc.dram_tensor("wc", (C, C), BF16, kind="ExternalInput")
    sc_d = nc.dram_tensor("sc", (P, NT), F32, kind="ExternalInput")
    y_d = nc.dram_tensor("y", (CHUNK, C), BF16, kind="ExternalOutput")

    xt_ap, wc_ap, sc_ap, y_ap = xt_d.ap(), wc_d.ap(), sc_d.ap(), y_d.ap()

    with tile.TileContext(nc) as tc:
        with (
            tc.tile_pool(name="io", bufs=1) as io,
            tc.tile_pool(name="ps", bufs=1, space="PSUM") as ps_pool,
        ):
            # ---- input DMAs first: x feature-tiles on sync, Wc row-tiles
            # on scalar (two HWDGE rings in parallel), sc on gpsimd ----
            xts, wcs = [], []
            for i in range(NI):
                t = io.tile([P, CHUNK], BF16, name=f"xt{i}")
                nc.sync.dma_start(t[:], xt_ap[i * P : (i + 1) * P, :])
                xts.append(t)
            for i in range(NI):
                t = io.tile([P, C], BF16, name=f"wc{i}")
                nc.scalar.dma_start(t[:], wc_ap[i * P : (i + 1) * P, :])
                wcs.append(t)
            scs = io.tile([P, NT], F32, name="scs")
            nc.gpsimd.dma_start(scs[:], sc_ap[:, :])

            # ---- warmup: keep PE busy from t=0 so HAM unthrottles ----
            warm = io.tile([P, C], BF16, name="warm")
            nc.vector.memset(warm[:], 0.0)
            if n_warm:
                wps = ps_pool.tile([P, C], F32, name="wps", tag="warm")
                for _ in range(n_warm):
                    nc.tensor.matmul(
                        wps[:], warm[:, 0:P], warm[:], start=True, stop=True
                    )

            # ---- cumsum scans: A_i[f, t] = cumsum_t xT_i (DVE-only op) ----
            A_sb = [io.tile([P, CHUNK], BF16, name=f"A{i}") for i in range(NI)]
            for i in range(NI):
                nc.vector.tensor_tensor_scan(
                    A_sb[i][:], xts[i][:], xts[i][:], 0.0, ADD, BYPASS
                )

            # ---- stage M: psY_j += A_i[:, tile j]^T @ Wc rows i ----
            psY = [
                ps_pool.tile([P, C], F32, name=f"psY{j}", tag=f"Y{j}")
                for j in range(NT)
            ]
            for i in range(NI):
                for j in range(NT):
                    nc.tensor.matmul(
                        psY[j][:],
                        A_sb[i][:, j * P : (j + 1) * P],
                        wcs[i][:],
                        start=(i == 0),
                        stop=(i == NI - 1),
                    )
                    # as soon as psY_j is complete, evict (scaled) + DMA out
                    if i == NI - 1:
                        ysb = io.tile([P, C], BF16, name=f"y{j}")
                        if j % 2 == 0:
                            nc.scalar.mul(ysb[:], psY[j][:], scs[:, j : j + 1])
                        else:
                            nc.vector.tensor_scalar_mul(
                                ysb[:], psY[j][:], scs[:, j : j + 1]
                            )
                        deng = nc.sync if j % 2 == 0 else nc.scalar
                        deng.dma_start(y_ap[j * P : (j + 1) * P, :], ysb[:])

    nc.compile()
    return nc


def _get_nc():
    key = N_WARM[0]
    if key not in _STATE:
        _STATE[key] = _build_nc(key)
    return _STATE[key]


def _prepare_in_maps(x, w_attn, w_proj):
    x = np.asarray(x, dtype=np.float32)
    w_attn = np.asarray(w_attn, dtype=np.float32)
    w_proj = np.asarray(w_proj, dtype=np.float32)
    wc = (w_attn[:, 2 * C : 3 * C] @ w_proj).astype(BF16_NP)

    in_maps = []
    for core in range(N_CORES):
        b, tc_ = divmod(core, T // CHUNK)
        goff = tc_ * CHUNK
        xt = np.array(x[b, goff : goff + CHUNK, :].T, dtype=np.float32)
        if goff:
            # fold the carry into t=0 (cumsum then includes it everywhere)
            xt[:, 0] += x[b, :goff, :].sum(axis=0, dtype=np.float32)
        sc = (
            1.0 / (goff + 1 + np.arange(CHUNK, dtype=np.float32))
        ).reshape(NT, P).T.astype(np.float32)
        in_maps.append(
            {
                "xt": np.ascontiguousarray(xt).astype(BF16_NP),
                "wc": wc,
                "sc": np.ascontiguousarray(sc),
            }
        )
    return in_maps


def kernel(x, w_attn, w_proj):
    nc = _get_nc()
    in_maps = _prepare_in_maps(x, w_attn, w_proj)
    res = bass_utils.run_bass_kernel_spmd(
        nc, in_maps, core_ids=list(range(N_CORES)), trace=TRACE[0]
    )
    LAST_RESULT[0] = res
    y = np.empty((B, T, C), np.float32)
    for core in range(N_CORES):
        b, tc_ = divmod(core, T // CHUNK)
        y[b, tc_ * CHUNK : (tc_ + 1) * CHUNK, :] = res.results[core]["y"].astype(
            np.float32
        )
    return y


# revision 8
# speedup vs baseline: 1.6382x; 1.0232x over previous
"""Trainium2 Bass kernel for nn_CausalSelfAttention_74268574482879.

The reference module's attention scores are overwritten by the causal mask
(q/k are discarded), so softmax weights are uniform over positions <= t:
    y = cummean_T(x) @ W_v @ W_p

Host-side algebra (all exact up to fp rounding):
  * W_c = W_v @ W_p is folded into a single 512x512 weight.
  * The 4096 rows of (B*T) are split into 8 chunks of 512 rows, one per
    NeuronCore.  The cross-chunk carry (column-sum of all preceding rows in
    the same batch element) is added into column 0 of the transposed chunk
    on the host, so the device computes a plain local cumsum.
  * x is passed TRANSPOSED (feature-major) so the cumsum runs as a DVE /
    GpSimd ``tensor_tensor_scan`` along the free (time) dim — no PE work.
  * Everything is cast to bf16 on the host (rel-err budget is 2e-2).

Per-core dataflow:
  scan_i : A^T_i[f, t] = cumsum_t(xT_i[f, t])   (DVE/GpSimd, bf16 out)
  M      : psY_j = sum_i A_i[tile j]^T-slice @ W_c rows i   (16 matmuls)
  evict  : ysb_j = psY_j * 1/(t+1)  (per-partition scalar, DVE/ACT/GpSimd)
A few throwaway matmuls on memset data run during the initial DMA fill to
lift the PE HAM clock-gate early.
"""

import numpy as np
import ml_dtypes

import concourse.bass as bass
import concourse.bacc as bacc
import concourse.mybir as mybir
import concourse.tile as tile
from concourse import bass_utils

N_CORES = 8
B, T, C = 2, 2048, 512
CHUNK = 512               # rows of flattened (B*T) per core
P = 128
NT = CHUNK // P           # 4 row-tiles per chunk
NI = C // P               # 4 col-tiles of the 512 feature dim
F32 = mybir.dt.float32
BF16 = mybir.dt.bfloat16
BF16_NP = ml_dtypes.bfloat16
ADD = mybir.AluOpType.add
BYPASS = mybir.AluOpType.bypass

N_WARM = [8]              # warmup matmuls (HAM unthrottle) during DMA fill
TRACE = [False]
LAST_RESULT = [None]
_STATE = {}


def _build_nc(n_warm):
    nc = bacc.Bacc(
        "TRN2", target_bir_lowering=False, debug=False, num_devices=N_CORES
    )

    xt_d = nc.dram_tensor("xt", (C, CHUNK), BF16, kind="ExternalInput")
    wc_d = nc.dram_tensor("wc", (C, C), BF16, kind="ExternalInput")
    sc_d = nc.dram_tensor("sc", (P, NT), F32, kind="ExternalInput")
    y_d = nc.dram_tensor("y", (CHUNK, C), BF16, kind="ExternalOutput")

    xt_ap, wc_ap, sc_ap, y_ap = xt_d.ap(), wc_d.ap(), sc_d.ap(), y_d.ap()

    with tile.TileContext(nc) as tc:
        with (
            tc.tile_pool(name="io", bufs=1) as io,
            tc.tile_pool(name="ps", bufs=1, space="PSUM") as ps_pool,
        ):
            # ---- input DMAs first: x feature-tiles on sync, Wc row-tiles
            # on scalar (two HWDGE rings in parallel), sc on gpsimd.
            # xt_0/xt_1 get their own transfers (they gate the scan chain);
            # the rest ride in combined transfers to save DIRECT2D time ----
            xt0 = io.tile([P, CHUNK], BF16, name="xt0")
            nc.sync.dma_start(xt0[:], xt_ap[0:P, :])
            xt1 = io.tile([P, CHUNK], BF16, name="xt1")
            nc.sync.dma_start(xt1[:], xt_ap[P : 2 * P, :])
            xt23 = io.tile([P, 2, CHUNK], BF16, name="xt23")
            nc.sync.dma_start(
                xt23[:], xt_ap[2 * P :, :].rearrange("(i p) t -> p i t", p=P)
            )
            xts = [xt0, xt1, xt23[:, 0, :], xt23[:, 1, :]]
            wc0 = io.tile([P, C], BF16, name="wc0")
            nc.scalar.dma_start(wc0[:], wc_ap[0:P, :])
            wc123 = io.tile([P, NI - 1, C], BF16, name="wc123")
            nc.scalar.dma_start(
                wc123[:], wc_ap[P:, :].rearrange("(i p) c -> p i c", p=P)
            )
            wcs = [wc0] + [wc123[:, i, :] for i in range(NI - 1)]
            scs = io.tile([P, NT], F32, name="scs")
            nc.gpsimd.dma_start(scs[:], sc_ap[:, :])

            # ---- warmup: keep PE busy from t=0 so HAM unthrottles ----
            warm = io.tile([P, C], BF16, name="warm")
            nc.vector.memset(warm[:], 0.0)
            if n_warm:
                wps = ps_pool.tile([P, C], F32, name="wps", tag="warm")
                for _ in range(n_warm):
                    nc.tensor.matmul(
                        wps[:], warm[:, 0:P], warm[:], start=True, stop=True
                    )

            # ---- cumsum scans: A_i[f, t] = cumsum_t xT_i (DVE-only op) ----
            A_sb = [io.tile([P, CHUNK], BF16, name=f"A{i}") for i in range(NI)]
            for i in range(NI):
                nc.vector.tensor_tensor_scan(
                    A_sb[i][:], xts[i][:], xts[i][:], 0.0, ADD, BYPASS
                )

            # ---- stage M: psY_j += A_i[:, tile j]^T @ Wc rows i ----
            psY = [
                ps_pool.tile([P, C], F32, name=f"psY{j}", tag=f"Y{j}")
                for j in range(NT)
            ]
            for i in range(NI):
                for j in range(NT):
                    nc.tensor.matmul(
                        psY[j][:],
                        A_sb[i][:, j * P : (j + 1) * P],
                        wcs[i][:],
                        start=(i == 0),
                        stop=(i == NI - 1),
                    )
                    # as soon as psY_j is complete, evict (scaled) + DMA out
                    if i == NI - 1:
                        ysb = io.tile([P, C], BF16, name=f"y{j}")
                        if j % 2 == 0:
                            nc.scalar.mul(ysb[:], psY[j][:], scs[:, j : j + 1])
                        else:
                            nc.vector.tensor_scalar_mul(
                                ysb[:], psY[j][:], scs[:, j : j + 1]
                            )
                        deng = nc.sync if j % 2 == 0 else nc.scalar
                        deng.dma_start(y_ap[j * P : (j + 1) * P, :], ysb[:])

    nc.compile()
    return nc


def _get_nc():
    key = N_WARM[0]
    if key not in _STATE:
        _STATE[key] = _build_nc(key)
    return _STATE[key]


def _prepare_in_maps(x, w_attn, w_proj):
    x = np.asarray(x, dtype=np.float32)
    w_attn = np.asarray(w_attn, dtype=np.float32)
    w_proj = np.asarray(w_proj, dtype=np.float32)
    wc = (w_attn[:, 2 * C : 3 * C] @ w_proj).astype(BF16_NP)

    in_maps = []
    for core in range(N_CORES):
        b, tc_ = divmod(core, T // CHUNK)
        goff = tc_ * CHUNK
        xt = np.array(x[b, goff : goff + CHUNK, :].T, dtype=np.float32)
        if goff:
            # fold the carry into t=0 (cumsum then includes it everywhere)
            xt[:, 0] += x[b, :goff, :].sum(axis=0, dtype=np.float32)
        sc = (
            1.0 / (goff + 1 + np.arange(CHUNK, dtype=np.float32))
        ).reshape(NT, P).T.astype(np.float32)
        in_maps.append(
            {
                "xt": np.ascontiguousarray(xt).astype(BF16_NP),
                "wc": wc,
                "sc": np.ascontiguousarray(sc),
            }
        )
    return in_maps


def kernel(x, w_attn, w_proj):
    nc = _get_nc()
    in_maps = _prepare_in_maps(x, w_attn, w_proj)
    res = bass_utils.run_bass_kernel_spmd(
        nc, in_maps, core_ids=list(range(N_CORES)), trace=TRACE[0]
    )
    LAST_RESULT[0] = res
    y = np.empty((B, T, C), np.float32)
    for core in range(N_CORES):
        b, tc_ = divmod(core, T // CHUNK)
        y[b, tc_ * CHUNK : (tc_ + 1) * CHUNK, :] = res.results[core]["y"].astype(
            np.float32
        )
    return y
